# revision 1
# baseline (speedup 1.0000x reference)
"""Trainium2 Bass kernel for nn_EnerG (3-layer NNConv GNN + sum-pool + MLP).

Strategy (8 cores, SPMD):
  - Edges sharded across cores (4000/core, padded to 4096), sorted by dst.
  - Node features replicated; per-layer aggregation via indirect-DMA-add
    scatter into DRAM, AllReduce across cores, replicated node update.
  - Per-edge weight tensors (the big [E,64,128] intermediate) are fused in
    SBUF: PE generates pre-activations, ACT applies leaky-relu on PSUM
    evacuation, DVE contracts against gathered source features.
"""
import sys

sys.path.insert(0, "/opt/trn_rl_repo")

import numpy as np

import concourse.bass as bass
import concourse.tile as tile
from concourse import bacc, mybir
from concourse.bass_utils import run_bass_kernel_spmd
from concourse.masks import make_identity

F32 = mybir.dt.float32
BF16 = mybir.dt.bfloat16
I32 = mybir.dt.int32
AF = mybir.ActivationFunctionType
OP = mybir.AluOpType

N_CORES = 8
N = 8000
E = 32000
G = 32
NP = 8064            # padded nodes (63 * 128)
EC = 4096            # padded edges per core (32 * 128)
NT = NP // 128       # 63 node tiles
ET = EC // 128       # 32 edge tiles
JUNK = NP - 1        # junk node for dead scatter slots

# layer dims: (in_ch, out_ch)
LAYERS = [(4, 8), (8, 64), (64, 128)]


def build_graph(reps: int = 1, debug: bool = False, ablate=()):
    nc = bacc.Bacc("TRN2", target_bir_lowering=False, debug=False,
                   num_devices=N_CORES)

    def din(name, shape, dt=F32):
        return nc.dram_tensor(name, shape, dt, kind="ExternalInput").ap()

    # per-core edge data
    eaT = din("eaT", [4, EC])                 # edge attrs ^T + ones row
    xsrc = din("xsrc", [EC, 4])               # x[src] (layer-1 H)
    srcidx = din("srcidx", [EC, 1], I32)
    dstw = din("dstw", [EC, 1], I32)          # dedup'd dst (else JUNK)
    mcomb = din("mcomb", [128, EC])           # per-tile combine matrices
    # replicated node/graph data
    xaT = din("xaT", [5, NP])                 # x^T + ones row
    batchg = din("batchg", [NP, 1], I32)
    # weights
    w1a = [din(f"w1a{l}", [4, 64]) for l in range(3)]
    w2a = [din(f"w2a{l}", [65, LAYERS[l][0] * LAYERS[l][1] + 0]) for l in range(3)]
    r_a = [din("r1a", [5, 8]), din("r2a", [9, 64]), din("r3a", [65, 128])]
    fc1 = din("fc1", [128, 128])
    fc1b = din("fc1b", [128, 1])
    fc2 = din("fc2", [128, 64])
    fc2b = din("fc2b", [64, 1])
    fc3 = din("fc3", [64, 1])
    fc3b = din("fc3b", [32, 1])

    out = nc.dram_tensor("out", [G, 1], F32, kind="ExternalOutput").ap()
    dbg = {}
    if debug:
        dbg["efT1"] = nc.dram_tensor("efT1_o", [65, EC], F32, kind="ExternalOutput").ap()
        for l in range(3):
            dbg[f"msgs{l}"] = nc.dram_tensor(f"msgs{l}_o", [EC, LAYERS[l][1]], F32,
                                             kind="ExternalOutput").ap()
            dbg[f"aggl{l}"] = nc.dram_tensor(f"aggl{l}_o", [NP, LAYERS[l][1]], F32,
                                             kind="ExternalOutput").ap()
        dbg["h1"] = nc.dram_tensor("h1_o", [NP, 8], F32, kind="ExternalOutput").ap()
        dbg["h2"] = nc.dram_tensor("h2_o", [NP, 64], F32, kind="ExternalOutput").ap()
        dbg["pool"] = nc.dram_tensor("pool_o", [128, G], F32, kind="ExternalOutput").ap()

    # internal DRAM
    h_dram = [None,
              nc.dram_tensor("h1d", [NP, 8], F32).ap(),
              nc.dram_tensor("h2d", [NP, 64], F32).ap()]
    agg_l = [nc.dram_tensor(f"agg{l}", [NP, LAYERS[l][1]], F32).ap()
             for l in range(3)]
    hpT_dram = [None,
                nc.dram_tensor("h1pTd", [9, NP], F32).ap(),
                nc.dram_tensor("h2pTd", [65, NP], F32).ap()]
    agg_g = [nc.dram_tensor(f"aggg{l}", [NP, LAYERS[l][1]], F32,
                            addr_space="Shared").ap() for l in range(3)]
    pool_l = nc.dram_tensor("pooll", [128, G], F32).ap()
    pool_g = nc.dram_tensor("poolg", [128, G], F32, addr_space="Shared").ap()

    groups = [list(range(N_CORES))]

    with tile.TileContext(nc) as tc:
        _build_body(nc, tc, reps, locals(), dbg, ablate)
    nc.compile()
    return nc


def _build_body(nc, tc, reps, v, dbg=None, ablate=()):
    dbg = dbg or {}
    eaT, xsrc, srcidx, dstw, mcomb = v["eaT"], v["xsrc"], v["srcidx"], v["dstw"], v["mcomb"]
    xaT, batchg = v["xaT"], v["batchg"]
    w1a, w2a, r_a = v["w1a"], v["w2a"], v["r_a"]
    fc1, fc1b, fc2, fc2b, fc3, fc3b = v["fc1"], v["fc1b"], v["fc2"], v["fc2b"], v["fc3"], v["fc3b"]
    out, h_dram, agg_l, agg_g = v["out"], v["h_dram"], v["agg_l"], v["agg_g"]
    hpT_dram = v["hpT_dram"]
    pool_l, pool_g, groups = v["pool_l"], v["pool_g"], v["groups"]

    import contextlib
    ctx = contextlib.ExitStack()
    with ctx:
        persist = ctx.enter_context(tc.tile_pool(name="persist", bufs=1))
        wpool = ctx.enter_context(tc.tile_pool(name="wpool", bufs=2))
        small = ctx.enter_context(tc.tile_pool(name="small", bufs=4))
        psum_w = ctx.enter_context(tc.tile_pool(name="psum_w", bufs=2, space="PSUM"))
        psum_m = ctx.enter_context(tc.tile_pool(name="psum_m", bufs=4, space="PSUM"))

        # ---- persistent SBUF loads ----
        ident = persist.tile([128, 128], F32, name="ident")
        make_identity(nc, ident[:])
        ident_bf = persist.tile([128, 128], BF16, name="ident_bf")
        nc.vector.tensor_copy(ident_bf[:], ident[:])
        mcomb_sb = persist.tile([128, EC], F32, name="mcomb_sb")
        nc.sync.dma_start(mcomb_sb[:], mcomb[:])
        w1a_sb = []
        w2a_sb = []
        for l in range(3):
            t1 = persist.tile([4, 64], F32, name=f"w1a_sb{l}")
            nc.sync.dma_start(t1[:], w1a[l][:])
            w1a_sb.append(t1)
            c = LAYERS[l][0] * LAYERS[l][1]
            t2 = persist.tile([65, c], F32, name=f"w2a_sb{l}")
            nc.sync.dma_start(t2[:], w2a[l][:])
            w2a_sb.append(t2)
        ra_sb = []
        for l, shp in enumerate([[5, 8], [9, 64], [65, 128]]):
            t = persist.tile(shp, F32, name=f"ra_sb{l}")
            nc.sync.dma_start(t[:], r_a[l][:])
            ra_sb.append(t)
        fc1_sb = persist.tile([128, 128], F32, name="fc1_sb")
        nc.sync.dma_start(fc1_sb[:], fc1[:])
        fc1b_sb = persist.tile([128, 1], F32, name="fc1b_sb")
        nc.sync.dma_start(fc1b_sb[:], fc1b[:])
        fc2_sb = persist.tile([128, 64], F32, name="fc2_sb")
        nc.sync.dma_start(fc2_sb[:], fc2[:])
        fc2b_sb = persist.tile([64, 1], F32, name="fc2b_sb")
        nc.sync.dma_start(fc2b_sb[:], fc2b[:])
        fc3_sb = persist.tile([64, 1], F32, name="fc3_sb")
        nc.sync.dma_start(fc3_sb[:], fc3[:])
        fc3b_sb = persist.tile([32, 1], F32, name="fc3b_sb")
        nc.sync.dma_start(fc3b_sb[:], fc3b[:])
        zero_sb = persist.tile([128, 128], F32, name="zero_sb")
        nc.vector.memset(zero_sb[:], 0.0)
        iota_f = persist.tile([128, 128], F32, name="iota_f")
        iota_i = persist.tile([128, 128], I32, name="iota_i")
        nc.gpsimd.iota(iota_i[:], pattern=[[1, 128]], channel_multiplier=0)
        nc.vector.tensor_copy(iota_f[:], iota_i[:])

        # gathered H per layer lives here ([128, 32*in_ch])
        h_gath = persist.tile([128, ET * 64], F32, name="h_gath")
        # index tiles: column t holds tile t's rows
        srcidx_sb = persist.tile([128, ET], I32, name="srcidx_sb")
        nc.sync.dma_start(srcidx_sb[:], srcidx[:].rearrange("(t p) one -> p (t one)", p=128))
        dstw_sb = persist.tile([128, ET], I32, name="dstw_sb")
        nc.sync.dma_start(dstw_sb[:], dstw[:].rearrange("(t p) one -> p (t one)", p=128))
        efT = persist.tile([65, EC], F32, name="efT")

        for rep in range(reps):
            poolT_ps = None
            for l in range(3):
                for j in range(NT):
                    nc.sync.dma_start(agg_l[l][128 * j:128 * (j + 1), :],
                                      zero_sb[:, :LAYERS[l][1]])
            for l in range(3):
                cin, cout = LAYERS[l]
                ncols = cin * cout

                # ---- edge-net hidden: efT = Prelu(w1a^T @ eaT), + ones row
                for ch in range(EC // 512):
                    ea_t = small.tile([4, 512], F32, name="ea_t")
                    nc.sync.dma_start(ea_t[:], eaT[:, 512 * ch:512 * (ch + 1)])
                    ps = psum_m.tile([128, 512], F32, name="efps", tag="psm")
                    nc.tensor.matmul(ps[:64, :], lhsT=w1a_sb[l][:],
                                     rhs=ea_t[:], start=True, stop=True)
                    nc.scalar.activation(efT[0:64, 512 * ch:512 * (ch + 1)],
                                         ps[:64, :], AF.Prelu, alpha=0.1)
                nc.vector.memset(efT[64:65, :], 1.0)
                if l == 0 and "efT1" in dbg:
                    for ch in range(EC // 512):
                        sn = small.tile([65, 512], F32, name="efsn")
                        nc.vector.tensor_copy(sn[:], efT[:, 512 * ch:512 * (ch + 1)])
                        nc.sync.dma_start(dbg["efT1"][:, 512 * ch:512 * (ch + 1)], sn[:])

                # ---- gather H (source features) ----
                if l == 0:
                    for t in range(ET):
                        nc.sync.dma_start(h_gath[:, 64 * t:64 * t + cin],
                                          xsrc[128 * t:128 * (t + 1), :])
                else:
                    for t in range(ET):
                        if "nogath" in ablate:
                            nc.sync.dma_start(h_gath[:, 64 * t:64 * t + cin],
                                              h_dram[l][:128, :][0:128, :])
                            continue
                        nc.gpsimd.indirect_dma_start(
                            out=h_gath[:, 64 * t:64 * t + cin],
                            out_offset=None,
                            in_=h_dram[l][:],
                            in_offset=bass.IndirectOffsetOnAxis(
                                ap=srcidx_sb[:, t:t + 1], axis=0),
                        )

                # ---- edge stream ----
                for t in range(ET):
                    acc = small.tile([128, 128], F32, name="acc")
                    hsl = h_gath[:, 64 * t:64 * t + cin]
                    if l == 2:
                        for half in range(2):
                            base = 4096 * half
                            w_sb = wpool.tile([128, 4096], F32, name="w_sb")
                            for gb in range(0, 4096, 1024):
                                ps = psum_w.tile([128, 1024], F32, name="wps")
                                for sb in range(0, 1024, 512):
                                    nc.tensor.matmul(
                                        ps[:, sb:sb + 512],
                                        lhsT=efT[:, 128 * t:128 * (t + 1)],
                                        rhs=w2a_sb[l][:, base + gb + sb:
                                                       base + gb + sb + 512],
                                        start=True, stop=True)
                                nc.scalar.activation(w_sb[:, gb:gb + 1024],
                                                     ps[:], AF.Prelu,
                                                     alpha=0.1)
                            for i in range(32 * half, 32 * half + 32):
                                src_ap = w_sb[:, i * cout - base:
                                              (i + 1) * cout - base]
                                if i == 0:
                                    nc.vector.tensor_scalar(
                                        acc[:, :cout], src_ap, hsl[:, 0:1],
                                        None, op0=OP.mult)
                                else:
                                    nc.vector.scalar_tensor_tensor(
                                        out=acc[:, :cout], in0=src_ap,
                                        scalar=hsl[:, i:i + 1],
                                        in1=acc[:, :cout],
                                        op0=OP.mult, op1=OP.add)
                    else:
                        w_sb = wpool.tile([128, 4096], F32, name="w_sbf")
                        # W-gen (PE) + leaky evacuation (ACT)
                        for gb in range(0, ncols, 1024):
                            gcols = min(1024, ncols - gb)
                            ps = psum_w.tile([128, 1024], F32, name="wps")
                            for sb in range(0, gcols, 512):
                                scols = min(512, gcols - sb)
                                nc.tensor.matmul(
                                    ps[:, sb:sb + scols],
                                    lhsT=efT[:, 128 * t:128 * (t + 1)],
                                    rhs=w2a_sb[l][:, gb + sb:gb + sb + scols],
                                    start=True, stop=True)
                            nc.scalar.activation(w_sb[:, gb:gb + gcols],
                                                 ps[:, :gcols], AF.Prelu,
                                                 alpha=0.1)
                        # MAC (DVE): acc[e, o] += W[e, i*cout+o] * H[e, i]
                        for i in range(cin):
                            src_ap = w_sb[:, i * cout:(i + 1) * cout]
                            if i == 0:
                                nc.vector.tensor_scalar(
                                    acc[:, :cout], src_ap, hsl[:, 0:1], None,
                                    op0=OP.mult)
                            else:
                                nc.vector.scalar_tensor_tensor(
                                    out=acc[:, :cout], in0=src_ap,
                                    scalar=hsl[:, i:i + 1], in1=acc[:, :cout],
                                    op0=OP.mult, op1=OP.add)

                    if f"msgs{l}" in dbg:
                        nc.sync.dma_start(dbg[f"msgs{l}"][128 * t:128 * (t + 1), :],
                                          acc[:, :cout])
                    # combine duplicates within tile + scatter-add to DRAM
                    cps = psum_m.tile([128, 512], F32, name="cps", tag="psm")
                    nc.tensor.matmul(cps[:, :cout],
                                     lhsT=mcomb_sb[:, 128 * t:128 * (t + 1)],
                                     rhs=acc[:, :cout], start=True, stop=True)
                    msg2 = small.tile([128, 128], F32, name="msg2")
                    nc.vector.tensor_copy(msg2[:, :cout], cps[:, :cout])
                    if "noscat" not in ablate:
                        nc.gpsimd.indirect_dma_start(
                            out=agg_l[l][:],
                            out_offset=bass.IndirectOffsetOnAxis(
                                ap=dstw_sb[:, t:t + 1], axis=0),
                            in_=msg2[:, :cout], in_offset=None,
                            compute_op=OP.add)
                    else:
                        nc.sync.dma_start(
                            agg_l[l][128 * t:128 * (t + 1), :],
                            msg2[:, :cout])

                if f"aggl{l}" in dbg:
                    for j in range(NT):
                        sn = small.tile([128, 128], F32, name="aggsn")
                        nc.sync.dma_start(sn[:, :cout],
                                          agg_l[l][128 * j:128 * (j + 1), :])
                        nc.sync.dma_start(dbg[f"aggl{l}"][128 * j:128 * (j + 1), :],
                                          sn[:, :cout])
                # ---- AllReduce ----
                if "nocoll" not in ablate:
                    nc.gpsimd.collective_compute(
                        "AllReduce", OP.add, replica_groups=groups,
                        ins=[agg_l[l][:]], outs=[agg_g[l][:]])

                # ---- node update: h = Prelu(agg + h_prev' @ root') ----
                prevT_dram = xaT if l == 0 else hpT_dram[l]
                kdim = [5, 9, 65][l]
                for j in range(NT):
                    pvT = small.tile([65, 128], F32, name="pvT")
                    nc.sync.dma_start(pvT[:kdim, :],
                                      prevT_dram[:, 128 * j:128 * (j + 1)])
                    ps = psum_m.tile([128, 512], F32, name="hups", tag="psm")
                    nc.tensor.matmul(ps[:, :cout],
                                     lhsT=pvT[:kdim, :],
                                     rhs=ra_sb[l][:], start=True, stop=False)
                    ag = small.tile([128, 128], F32, name="ag")
                    agsrc = agg_l[l] if "nocoll" in ablate else agg_g[l]
                    nc.sync.dma_start(ag[:, :cout],
                                      agsrc[128 * j:128 * (j + 1), :])
                    nc.tensor.matmul(ps[:, :cout], lhsT=ident[:],
                                     rhs=ag[:, :cout], start=False, stop=True)
                    if l < 2:
                        # h tile (+ones col), write DRAM + build transposed form
                        ht = small.tile([128, 72], F32, name="ht")
                        nc.scalar.activation(ht[:, :cout], ps[:, :cout],
                                             AF.Prelu, alpha=0.1)
                        nc.sync.dma_start(h_dram[l + 1][128 * j:128 * (j + 1), :],
                                          ht[:, :cout])
                        if f"h{l + 1}" in dbg:
                            nc.sync.dma_start(
                                dbg[f"h{l + 1}"][128 * j:128 * (j + 1), :],
                                ht[:, :cout])
                        nc.vector.memset(ht[:, cout:cout + 1], 1.0)
                        tps = psum_m.tile([128, 512], F32, name="tps", tag="psm")
                        nc.tensor.transpose(tps[:cout + 1, :128],
                                            ht[:, :cout + 1], ident[:])
                        hT_sb = small.tile([65, 128], F32, name="hT_sb")
                        nc.vector.tensor_copy(hT_sb[:cout + 1, :],
                                              tps[:cout + 1, :128])
                        nc.sync.dma_start(
                            hpT_dram[l + 1][:, 128 * j:128 * (j + 1)],
                            hT_sb[:cout + 1, :])
                    else:
                        # layer 3: h3 tile feeds pooling directly
                        ht = small.tile([128, 128], F32, name="ht3")
                        nc.scalar.activation(ht[:, :cout], ps[:, :cout],
                                             AF.Prelu, alpha=0.1)
                        bg = small.tile([128, 1], I32, name="bg")
                        nc.sync.dma_start(bg[:], batchg[128 * j:128 * (j + 1), :])
                        bgf = small.tile([128, 1], F32, name="bgf")
                        nc.vector.tensor_copy(bgf[:], bg[:])
                        oh = small.tile([128, G], F32, name="oh")
                        nc.vector.tensor_scalar(oh[:], iota_f[:, :G], bgf[:, :1],
                                                None, op0=OP.is_equal)
                        if j == 0:
                            poolT_ps = psum_m.tile([128, 512], F32, name="poolps", tag="psm")
                        nc.tensor.matmul(poolT_ps[:, :G], lhsT=ht[:, :cout],
                                         rhs=oh[:], start=(j == 0),
                                         stop=(j == NT - 1))

            # ---- MLP (all transposed; pool already complete per-core) ----
            pg = small.tile([128, G], F32, name="pg")
            nc.vector.tensor_copy(pg[:], poolT_ps[:, :G])
            if "pool" in dbg:
                nc.sync.dma_start(dbg["pool"][:], pg[:])

            m1 = psum_m.tile([128, 512], F32, name="m1", tag="psm")
            nc.tensor.matmul(m1[:, :G], lhsT=fc1_sb[:], rhs=pg[:], start=True, stop=True)
            t1 = small.tile([128, G], F32, name="t1")
            nc.scalar.activation(t1[:], m1[:, :G], AF.Prelu, bias=fc1b_sb[:, :1],
                                 alpha=0.1)
            m2 = psum_m.tile([128, 512], F32, name="m2", tag="psm")
            nc.tensor.matmul(m2[:64, :G], lhsT=fc2_sb[:], rhs=t1[:], start=True, stop=True)
            t2 = small.tile([64, G], F32, name="t2")
            nc.scalar.activation(t2[:], m2[:64, :G], AF.Prelu, bias=fc2b_sb[:, :1],
                                 alpha=0.1)
            # final: out[g] = t2[:, g] . fc3 + b  (transposed matmul -> [32, 1])
            m3 = psum_m.tile([128, 512], F32, name="m3", tag="psm")
            nc.tensor.matmul(m3[:G, :1], lhsT=t2[:], rhs=fc3_sb[:], start=True, stop=True)
            ot = small.tile([G, 1], F32, name="ot")
            nc.vector.scalar_tensor_tensor(out=ot[:], in0=m3[:G, :1], scalar=1.0,
                                           in1=fc3b_sb[:G, :], op0=OP.mult,
                                           op1=OP.add)
            nc.sync.dma_start(out[:], ot[:])


# ---------------------------------------------------------------------------
# host side
# ---------------------------------------------------------------------------

_CACHE = {}


def _prep_core_inputs(x, edge_index, batch_index, p):
    """Build per-core + replicated input arrays."""
    ec = E // N_CORES
    xs = x.astype(np.float32)
    xaT = np.zeros((5, NP), np.float32)
    xaT[:4, :N] = xs.T
    xaT[4, :] = 1.0
    bg = np.full((NP, 1), 64, np.int32)
    bg[:N, 0] = batch_index.astype(np.int32)

    def aug_w(w, b):
        return np.vstack([w, b[None, :]]).astype(np.float32)

    rep = dict(
        xaT=xaT, batchg=bg,
        w1a0=aug_w(p["en1_w1"], p["en1_b1"]), w2a0=aug_w(p["en1_w2"], p["en1_b2"]),
        w1a1=aug_w(p["en2_w1"], p["en2_b1"]), w2a1=aug_w(p["en2_w2"], p["en2_b2"]),
        w1a2=aug_w(p["en3_w1"], p["en3_b1"]), w2a2=aug_w(p["en3_w2"], p["en3_b2"]),
        r1a=aug_w(p["root1"], p["cb1"]), r2a=aug_w(p["root2"], p["cb2"]),
        r3a=aug_w(p["root3"], p["cb3"]),
        fc1=p["fc1_w"].astype(np.float32),
        fc1b=p["fc1_b"].reshape(128, 1).astype(np.float32),
        fc2=p["fc2_w"].astype(np.float32),
        fc2b=p["fc2_b"].reshape(64, 1).astype(np.float32),
        fc3=p["fc3_w"].astype(np.float32),
        fc3b=np.repeat(p["fc3_b"].reshape(1, 1), G, 0).astype(np.float32),
    )

    in_maps = []
    for c in range(N_CORES):
        sl = slice(c * ec, (c + 1) * ec)
        src = edge_index[0, sl].astype(np.int64)
        dst = edge_index[1, sl].astype(np.int64)
        order = np.argsort(dst, kind="stable")
        src, dst = src[order], dst[order]
        nreal = len(src)

        ea = (xs[dst] - xs[src])[:, 1:]                     # [ec, 3]
        eaT = np.zeros((4, EC), np.float32)
        eaT[:3, :nreal] = ea.T
        eaT[3, :] = 1.0
        xsrc = np.zeros((EC, 4), np.float32)
        xsrc[:nreal] = xs[src]
        srcidx = np.zeros((EC, 1), np.int32)
        srcidx[:nreal, 0] = src

        dstw = np.full((EC, 1), JUNK, np.int32)
        mcomb = np.zeros((128, EC), np.float32)
        for t in range(ET):
            lo = t * 128
            d_tile = dst[lo:min(lo + 128, nreal)] if lo < nreal else np.array([])
            first = {}
            for i, d in enumerate(d_tile):
                if d in first:
                    mcomb[i, lo + first[d]] = 1.0
                else:
                    first[d] = i
                    mcomb[i, lo + i] = 1.0
                    dstw[lo + i, 0] = d
        in_maps.append(dict(eaT=eaT, xsrc=xsrc, srcidx=srcidx, dstw=dstw,
                            mcomb=mcomb, **rep))
    return in_maps


def kernel(x, edge_index, batch_index, **p):
    if "nc" not in _CACHE:
        _CACHE["nc"] = build_graph(reps=1)
    nc = _CACHE["nc"]
    in_maps = _prep_core_inputs(np.asarray(x), np.asarray(edge_index),
                                np.asarray(batch_index),
                                {k: np.asarray(v) for k, v in p.items()})
    res = run_bass_kernel_spmd(nc, in_maps, list(range(N_CORES)))
    return res.results[0]["out"].astype(np.float32)



# revision 4
# speedup vs baseline: 122.7049x; 122.7049x over previous
"""Trainium2 Bass kernel for nn_EnerG (3-layer NNConv GNN + sum-pool + MLP).

v2 design (8 cores, SPMD, collective-free):
  - Each core redundantly computes the FULL graph (no collectives — they
    cannot live inside hardware loops, and instruction-stream overhead
    dominates this environment, so loops beat sharding).
  - Edges sorted by dst and packed host-side: node tile j (128 nodes) gets
    edge tiles 5j..5j+4 (640 slots).  Aggregation = per-tile P-matrix
    matmuls accumulated in PSUM — no scatter, no agg DRAM, no zeroing.
  - Per-rep body and per-layer node-tile loops are tc.For_i hardware
    loops, so the NEFF stays ~1.5K instructions regardless of reps.
  - W-gen matmuls run as float32r (full fp32 data, 4x PE rate).
  - PSUM evacuation of the per-edge weight tensor runs on ACT (Prelu,
    bf16 out); the per-edge contraction is a broadcast-multiply +
    segmented reduce on DVE (2 ops per tile).
"""
import contextlib
import sys

sys.path.insert(0, "/opt/trn_rl_repo")

import numpy as np

import concourse.bass as bass
import concourse.tile as tile
from concourse import bacc, mybir
from concourse.bass import ds
from concourse.bass_utils import run_bass_kernel_spmd
from concourse.masks import make_identity

F32 = mybir.dt.float32
F32R = mybir.dt.float32r
BF16 = mybir.dt.bfloat16
I32 = mybir.dt.int32
AF = mybir.ActivationFunctionType
OP = mybir.AluOpType

N_CORES = 8
N = 8000
E = 32000
G = 32
NP = 8192            # padded nodes (64 * 128)
NT = NP // 128       # 64 node tiles
K = 5                # edge tiles per node tile
ET = NT * K          # 320 edge tiles
EP = ET * 128        # 40960 edge slots
JUNK = NP - 1        # junk node for pad-edge sources

LAYERS = [(4, 8), (8, 64), (64, 128)]


def build_graph(reps: int = 1, debug: bool = False):
    nc = bacc.Bacc("TRN2", target_bir_lowering=False, debug=False,
                   num_devices=N_CORES)

    def din(name, shape, dt=F32):
        return nc.dram_tensor(name, shape, dt, kind="ExternalInput").ap()

    v = {}
    # per-node-tile bf16 stream: [xsrc(20) | dstloc(5) | onehot(32)] = 57 cols
    SW = 57
    v["strm"] = din("strm", [128, NT * SW], BF16)
    v["ea"] = din("ea", [4, EP])                 # edge attrs^T + ones row
    v["srcidx"] = din("srcidx", [128, ET], I32)  # src ids, tile t in col t
    v["xaT"] = din("xaT", [5, NP])               # x^T + ones row
    v["w1a"] = [din(f"w1a{l}", [4, 64]) for l in range(3)]
    v["w2a"] = [din(f"w2a{l}", [65, LAYERS[l][0] * LAYERS[l][1]], F32R)
                for l in range(3)]
    v["r_a"] = [din("r1a", [5, 8]), din("r2a", [9, 64]), din("r3a", [65, 128])]
    v["fc1"] = din("fc1", [128, 128])
    v["fc1b"] = din("fc1b", [128, 1])
    v["fc2"] = din("fc2", [128, 64])
    v["fc2b"] = din("fc2b", [64, 1])
    v["fc3"] = din("fc3", [64, 1])
    v["fc3b"] = din("fc3b", [G, 1])

    v["out"] = nc.dram_tensor("out", [G, 1], F32, kind="ExternalOutput").ap()
    dbg = {}
    if debug:
        dbg["h1"] = nc.dram_tensor("h1_o", [NP, 8], F32, kind="ExternalOutput").ap()
        dbg["h2"] = nc.dram_tensor("h2_o", [NP, 64], F32, kind="ExternalOutput").ap()
        dbg["pool"] = nc.dram_tensor("pool_o", [128, G], F32, kind="ExternalOutput").ap()

    # internal DRAM
    v["h_dram"] = [None,
                   nc.dram_tensor("h1d", [NP, 8], BF16).ap(),
                   nc.dram_tensor("h2d", [NP, 64], BF16).ap()]
    v["hpT_dram"] = [None,
                     nc.dram_tensor("h1pTd", [9, NP], F32).ap(),
                     nc.dram_tensor("h2pTd", [65, NP], F32).ap()]

    with tile.TileContext(nc) as tc:
        _build_body(nc, tc, reps, v, dbg)
    nc.compile()
    return nc


def _build_body(nc, tc, reps, v, dbg):
    ctx = contextlib.ExitStack()
    with ctx:
        persist = ctx.enter_context(tc.tile_pool(name="persist", bufs=1))
        psum_w = ctx.enter_context(tc.tile_pool(name="psum_w", bufs=3, space="PSUM"))
        psum_a = ctx.enter_context(tc.tile_pool(name="psum_a", bufs=1, space="PSUM"))
        psum_m = ctx.enter_context(tc.tile_pool(name="psum_m", bufs=2, space="PSUM"))

        # ---- persistent SBUF ----
        ident = persist.tile([128, 128], F32, name="ident")
        make_identity(nc, ident[:])
        w1a_sb, w2a_sb, ra_sb = [], [], []
        for l in range(3):
            t1 = persist.tile([4, 64], F32, name=f"w1a_sb{l}")
            nc.sync.dma_start(t1[:], v["w1a"][l][:])
            w1a_sb.append(t1)
            c = LAYERS[l][0] * LAYERS[l][1]
            t2 = persist.tile([65, c], F32R, name=f"w2a_sb{l}")
            nc.sync.dma_start(t2[:], v["w2a"][l][:])
            w2a_sb.append(t2)
            kd = LAYERS[l][0] + 1
            t3 = persist.tile([kd, LAYERS[l][1]], F32, name=f"ra_sb{l}")
            nc.sync.dma_start(t3[:], v["r_a"][l][:])
            ra_sb.append(t3)
        fc1_sb = persist.tile([128, 128], F32, name="fc1_sb")
        nc.sync.dma_start(fc1_sb[:], v["fc1"][:])
        fc1b_sb = persist.tile([128, 1], F32, name="fc1b_sb")
        nc.sync.dma_start(fc1b_sb[:], v["fc1b"][:])
        fc2_sb = persist.tile([128, 64], F32, name="fc2_sb")
        nc.sync.dma_start(fc2_sb[:], v["fc2"][:])
        fc2b_sb = persist.tile([64, 1], F32, name="fc2b_sb")
        nc.sync.dma_start(fc2b_sb[:], v["fc2b"][:])
        fc3_sb = persist.tile([64, 1], F32, name="fc3_sb")
        nc.sync.dma_start(fc3_sb[:], v["fc3"][:])
        fc3b_sb = persist.tile([G, 1], F32, name="fc3b_sb")
        nc.sync.dma_start(fc3b_sb[:], v["fc3b"][:])

        # working buffers, two sets (loop body handles 2 node tiles)
        ea_sb = [persist.tile([4, 640], F32, name=f"ea_sb{b}") for b in range(2)]
        efT = [persist.tile([65, 640], F32, name=f"efT{b}") for b in range(2)]
        nc.vector.memset(efT[0][64:65, :], 1.0)
        nc.vector.memset(efT[1][64:65, :], 1.0)
        efT_r = [persist.tile([65, 640], F32R, name=f"efT_r{b}") for b in range(2)]
        strm_sb = [persist.tile([128, 57], BF16, name=f"strm_sb{b}") for b in range(2)]
        pmat_sb = [persist.tile([128, 640], BF16, name=f"pmat_sb{b}") for b in range(2)]
        iota_i = persist.tile([128, 128], I32, name="iota_i")
        nc.gpsimd.iota(iota_i[:], pattern=[[1, 128]], channel_multiplier=0)
        iota_b = persist.tile([128, 128], BF16, name="iota_b")
        nc.vector.tensor_copy(iota_b[:], iota_i[:])
        sidx_sb = [persist.tile([128, K], I32, name=f"sidx_sb{b}") for b in range(2)]
        dst_f = [persist.tile([128, K], F32, name=f"dst_f{b}") for b in range(2)]
        hpT_sb = [persist.tile([65, 128], F32, name=f"hpT_sb{b}") for b in range(2)]
        h_gath = [persist.tile([128, K * 64], BF16, name=f"h_gath{b}") for b in range(2)]
        w_sb4 = [[persist.tile([128, 8192], BF16, name=f"w_sb{b}_{t}")
                  for t in range(2)] for b in range(2)]
        wh2 = [persist.tile([128, 8192], BF16, name=f"wh{b}")
               for b in range(2)]
        acc5 = [[persist.tile([128, 128], BF16, name=f"acc{b}_{t}")
                 for t in range(K)] for b in range(2)]
        hj = [persist.tile([128, 128], F32, name=f"hj{b}") for b in range(2)]
        hjb = [persist.tile([128, 128], BF16, name=f"hjb{b}") for b in range(2)]
        hT_sb = [persist.tile([65, 128], F32, name=f"hT_sb{b}") for b in range(2)]
        pool_sb = persist.tile([128, G], F32, name="pool_sb")
        mlp_t1 = persist.tile([128, G], F32, name="mlp_t1")
        mlp_t2 = persist.tile([64, G], F32, name="mlp_t2")
        mlp_ot = persist.tile([G, 1], F32, name="mlp_ot")

        h_dram, hpT_dram = v["h_dram"], v["hpT_dram"]

        def edge_tile(l, b, tt):
            """W-gen + MAC for one edge tile (o-major W)."""
            cin, cout = LAYERS[l]
            ncols = cin * cout
            w_sb = w_sb4[b][tt % 2]
            wh = wh2[b]
            acc = acc5[b][tt]
            ef_sl = efT_r[b][:, 128 * tt:128 * (tt + 1)]
            # W-gen: chunks of <=512 PSUM cols; Prelu evac to bf16
            for gb in range(0, ncols, 512):
                gcols = min(512, ncols - gb)
                ps = psum_w.tile([128, 512], F32, name="wps", tag="wps")
                nc.tensor.matmul(ps[:, :gcols],
                                 lhsT=ef_sl,
                                 rhs=w2a_sb[l][:, gb:gb + gcols],
                                 start=True, stop=True)
                nc.scalar.activation(w_sb[:, gb:gb + gcols], ps[:, :gcols],
                                     AF.Prelu, alpha=0.1)
            # MAC: wh = W (o-major) * h broadcast over o; reduce over i
            if l == 0:
                hsl = strm_sb[b][:, 4 * tt:4 * tt + cin]
            else:
                hsl = h_gath[b][:, 64 * tt:64 * tt + cin]
            w3 = w_sb[:, :ncols].rearrange("p (o i) -> p o i", i=cin)
            h3 = hsl.rearrange("p (o i) -> p o i", o=1) \
                    .broadcast_to([128, cout, cin])
            nc.vector.tensor_tensor(wh[:, :ncols].rearrange(
                "p (o i) -> p o i", i=cin), w3, h3, op=OP.mult)
            with nc.allow_low_precision("single rounding of fp32-accum sum"):
                nc.vector.tensor_reduce(
                    acc[:, :cout],
                    wh[:, :ncols].rearrange("p (o i) -> p o i", i=cin),
                    axis=mybir.AxisListType.X, op=OP.add)

        def sub_loads(l, b, j):
            """All DMAs for sub-tile j (issued on Pool queue)."""
            kd = LAYERS[l][0] + 1
            cin = LAYERS[l][0]
            nc.sync.dma_start(strm_sb[b][:], v["strm"][:, ds(57 * j, 57)])
            nc.sync.dma_start(ea_sb[b][:], v["ea"][:, ds(640 * j, 640)])
            if l == 0:
                nc.sync.dma_start(hpT_sb[b][:kd, :],
                                    v["xaT"][:, ds(128 * j, 128)])
            else:
                nc.sync.dma_start(sidx_sb[b][:],
                                    v["srcidx"][:, ds(K * j, K)])
                nc.sync.dma_start(hpT_sb[b][:kd, :],
                                    hpT_dram[l][:, ds(128 * j, 128)])
                for tt in range(K):
                    nc.gpsimd.indirect_dma_start(
                        out=h_gath[b][:, 64 * tt:64 * tt + cin],
                        out_offset=None,
                        in_=h_dram[l][:],
                        in_offset=bass.IndirectOffsetOnAxis(
                            ap=sidx_sb[b][:, tt:tt + 1], axis=0),
                    )

        def sub_compute(l, b, j):
            cin, cout = LAYERS[l]
            kd = cin + 1
            # build the 5 aggregation one-hot matrices from dstloc
            nc.vector.tensor_copy(dst_f[b][:], strm_sb[b][:, 20:25])
            for tt in range(K):
                nc.vector.tensor_scalar(
                    pmat_sb[b][:, 128 * tt:128 * (tt + 1)], iota_b[:],
                    dst_f[b][:, tt:tt + 1], None, op0=OP.is_equal)

            # edge-net hidden for the 5 tiles: efT = Prelu(w1a^T @ ea)
            for cs in range(0, 640, 512):
                cw = min(512, 640 - cs)
                psf = psum_m.tile([64, 512], F32, name="efps", tag="psm")
                nc.tensor.matmul(psf[:, :cw], lhsT=w1a_sb[l][:],
                                 rhs=ea_sb[b][:, cs:cs + cw],
                                 start=True, stop=True)
                nc.scalar.activation(efT[b][0:64, cs:cs + cw], psf[:, :cw],
                                     AF.Prelu, alpha=0.1)
            nc.vector.tensor_copy(efT_r[b][:], efT[b][:])

            # edge tiles: W-gen + MAC (PE runs all W-gens first)
            for tt in range(K):
                edge_tile(l, b, tt)
            # aggregation + root into one PSUM group
            agg_ps = psum_a.tile([128, 512], F32, name="agg_ps", tag="agg")
            for tt in range(K):
                nc.tensor.matmul(agg_ps[:, :cout],
                                 lhsT=pmat_sb[b][:, 128 * tt:128 * (tt + 1)],
                                 rhs=acc5[b][tt][:, :cout],
                                 start=(tt == 0), stop=False)
            # root term accumulated into the same PSUM group
            nc.tensor.matmul(agg_ps[:, :cout], lhsT=hpT_sb[b][:kd, :],
                             rhs=ra_sb[l][:], start=False, stop=True)

            # node update
            nc.scalar.activation(hj[b][:, :cout], agg_ps[:, :cout],
                                 AF.Prelu, alpha=0.1)
            if l < 2:
                nc.vector.tensor_copy(hjb[b][:, :cout], hj[b][:, :cout])
                nc.sync.dma_start(h_dram[l + 1][ds(128 * j, 128), :],
                                    hjb[b][:, :cout])
                nc.vector.memset(hj[b][:, cout:cout + 1], 1.0)
                tps = psum_m.tile([128, 512], F32, name="tps", tag="psm")
                nc.tensor.transpose(tps[:cout + 1, :128],
                                    hj[b][:, :cout + 1], ident[:])
                nc.vector.tensor_copy(hT_sb[b][:cout + 1, :],
                                      tps[:cout + 1, :128])
                nc.sync.dma_start(hpT_dram[l + 1][:, ds(128 * j, 128)],
                                    hT_sb[b][:cout + 1, :])
            else:
                nc.vector.tensor_copy(hjb[b][:, :cout], hj[b][:, :cout])
                pps = psum_m.tile([128, 512], F32, name="pps", tag="psm")
                nc.tensor.matmul(pps[:, :G], lhsT=hjb[b][:, :cout],
                                 rhs=strm_sb[b][:, 25:57], start=True,
                                 stop=True)
                nc.vector.tensor_tensor(pool_sb[:], pool_sb[:],
                                        pps[:, :G], op=OP.add)

        def layer_loop(l):
            with tc.For_i(0, NT // 2) as jh:
                for b in range(2):
                    sub_loads(l, b, 2 * jh + b)
                for b in range(2):
                    sub_compute(l, b, 2 * jh + b)

        rep_ctx = tc.For_i(0, reps)
        with rep_ctx:
            nc.vector.memset(pool_sb[:], 0.0)
            for l in range(3):
                layer_loop(l)

            if "pool" in dbg:
                nc.sync.dma_start(dbg["pool"][:], pool_sb[:])
            # ---- MLP (transposed) ----
            m1 = psum_m.tile([128, 512], F32, name="m1", tag="psm")
            nc.tensor.matmul(m1[:, :G], lhsT=fc1_sb[:], rhs=pool_sb[:],
                             start=True, stop=True)
            nc.scalar.activation(mlp_t1[:], m1[:, :G], AF.Prelu,
                                 bias=fc1b_sb[:, :1], alpha=0.1)
            m2 = psum_m.tile([128, 512], F32, name="m2", tag="psm")
            nc.tensor.matmul(m2[:64, :G], lhsT=fc2_sb[:], rhs=mlp_t1[:],
                             start=True, stop=True)
            nc.scalar.activation(mlp_t2[:], m2[:64, :G], AF.Prelu,
                                 bias=fc2b_sb[:, :1], alpha=0.1)
            m3 = psum_m.tile([128, 512], F32, name="m3", tag="psm")
            nc.tensor.matmul(m3[:G, :1], lhsT=mlp_t2[:], rhs=fc3_sb[:],
                             start=True, stop=True)
            nc.vector.scalar_tensor_tensor(out=mlp_ot[:], in0=m3[:G, :1],
                                           scalar=1.0, in1=fc3b_sb[:],
                                           op0=OP.mult, op1=OP.add)
            nc.sync.dma_start(v["out"][:], mlp_ot[:])

        if "h1" in dbg:
            snb = persist.tile([128, 64], BF16, name="dbg_snb")
            sn = persist.tile([128, 64], F32, name="dbg_sn")
            for jj in range(NT):
                nc.sync.dma_start(snb[:, :8], h_dram[1][128 * jj:128 * (jj + 1), :])
                nc.vector.tensor_copy(sn[:, :8], snb[:, :8])
                nc.sync.dma_start(dbg["h1"][128 * jj:128 * (jj + 1), :], sn[:, :8])
                nc.sync.dma_start(snb[:, :64], h_dram[2][128 * jj:128 * (jj + 1), :])
                nc.vector.tensor_copy(sn[:, :64], snb[:, :64])
                nc.sync.dma_start(dbg["h2"][128 * jj:128 * (jj + 1), :], sn[:, :64])


# ---------------------------------------------------------------------------
# host side
# ---------------------------------------------------------------------------

_CACHE = {}


def _prep_inputs(x, edge_index, batch_index, p):
    """Build the replicated input map (same for every core)."""
    xs = x.astype(np.float32)
    src_all = edge_index[0].astype(np.int64)
    dst_all = edge_index[1].astype(np.int64)

    # pack edges: node tile j gets slots [640j, 640j+cnt)
    ea = np.zeros((4, EP), np.float32)
    ea[3, :] = 1.0
    srcidx = np.full((EP,), JUNK, np.int64)
    dstloc = np.full((EP,), -1, np.int64)                   # local dst or -1
    eattr_all = (xs[dst_all] - xs[src_all])[:, 1:]          # [E, 3]

    tile_of_dst = dst_all // 128
    order = np.argsort(tile_of_dst, kind="stable")
    counts = np.bincount(tile_of_dst, minlength=NT)
    assert counts.max() <= K * 128, f"node tile overflow: {counts.max()}"
    starts = np.zeros(NT + 1, np.int64)
    starts[1:] = np.cumsum(counts)
    for j in range(NT):
        idx = order[starts[j]:starts[j + 1]]                 # edges of tile j
        n = len(idx)
        slots = 640 * j + np.arange(n)
        srcidx[slots] = src_all[idx]
        dstloc[slots] = dst_all[idx] - 128 * j
        ea[:3, slots] = eattr_all[idx].T

    srcidx_t = srcidx.reshape(ET, 128).T.astype(np.int32)    # [128, ET]
    xsrc = np.zeros((128, ET * 4), np.float32)
    xs_pad = np.zeros((NP, 4), np.float32)
    xs_pad[:N] = xs
    for t in range(ET):
        xsrc[:, 4 * t:4 * t + 4] = xs_pad[srcidx.reshape(ET, 128)[t]]

    import ml_dtypes
    bf16 = ml_dtypes.bfloat16

    onehot = np.zeros((128, NT * G), np.float32)
    bi = batch_index.astype(np.int64)
    for jj in range(NT):
        nodes = np.arange(128 * jj, 128 * jj + 128)
        real = nodes < N
        onehot[real, G * jj + bi[nodes[real]]] = 1.0

    # bf16 per-node-tile stream: [xsrc(20) | dstloc(5) | onehot(32)]
    strm = np.zeros((128, NT * 57), np.float32)
    dst_rt = dstloc.reshape(ET, 128)
    for j in range(NT):
        strm[:, 57 * j:57 * j + 20] = xsrc[:, 20 * j:20 * (j + 1)]
        for tt in range(K):
            strm[:, 57 * j + 20 + tt] = dst_rt[K * j + tt]
        strm[:, 57 * j + 25:57 * (j + 1)] = onehot[:, G * j:G * (j + 1)]

    xaT = np.zeros((5, NP), np.float32)
    xaT[:4, :N] = xs.T
    xaT[4, :] = 1.0

    def aug_w(w, b):
        return np.vstack([w, b[None, :]]).astype(np.float32)

    def om(w2, l):
        """Reorder edge-net output cols from i-major to o-major."""
        cin, cout = LAYERS[l]
        return np.ascontiguousarray(
            w2.reshape(-1, cin, cout).transpose(0, 2, 1).reshape(-1, cin * cout))

    return dict(
        ea=ea, strm=strm.astype(bf16), srcidx=srcidx_t, xaT=xaT,
        w1a0=aug_w(p["en1_w1"], p["en1_b1"]),
        w2a0=om(aug_w(p["en1_w2"], p["en1_b2"]), 0),
        w1a1=aug_w(p["en2_w1"], p["en2_b1"]),
        w2a1=om(aug_w(p["en2_w2"], p["en2_b2"]), 1),
        w1a2=aug_w(p["en3_w1"], p["en3_b1"]),
        w2a2=om(aug_w(p["en3_w2"], p["en3_b2"]), 2),
        r1a=aug_w(p["root1"], p["cb1"]), r2a=aug_w(p["root2"], p["cb2"]),
        r3a=aug_w(p["root3"], p["cb3"]),
        fc1=p["fc1_w"].astype(np.float32),
        fc1b=p["fc1_b"].reshape(128, 1).astype(np.float32),
        fc2=p["fc2_w"].astype(np.float32),
        fc2b=p["fc2_b"].reshape(64, 1).astype(np.float32),
        fc3=p["fc3_w"].astype(np.float32),
        fc3b=np.repeat(p["fc3_b"].reshape(1, 1), G, 0).astype(np.float32),
    )


def kernel(x, edge_index, batch_index, **p):
    if "nc" not in _CACHE:
        _CACHE["nc"] = build_graph(reps=1)
    nc = _CACHE["nc"]
    in_map = _prep_inputs(np.asarray(x), np.asarray(edge_index),
                          np.asarray(batch_index),
                          {k: np.asarray(v) for k, v in p.items()})
    in_maps = [in_map for _ in range(N_CORES)]
    res = run_bass_kernel_spmd(nc, in_maps, list(range(N_CORES)))
    return res.results[0]["out"].astype(np.float32)


# revision 8
# speedup vs baseline: 126.8946x; 1.0341x over previous
"""Trainium2 Bass kernel for nn_EnerG (3-layer NNConv GNN + sum-pool + MLP).

v2 design (8 cores, SPMD, collective-free):
  - Each core redundantly computes the FULL graph (no collectives — they
    cannot live inside hardware loops, and instruction-stream overhead
    dominates this environment, so loops beat sharding).
  - Edges sorted by dst and packed host-side: node tile j (128 nodes) gets
    edge tiles 5j..5j+4 (640 slots).  Aggregation = per-tile P-matrix
    matmuls accumulated in PSUM — no scatter, no agg DRAM, no zeroing.
  - Per-rep body and per-layer node-tile loops are tc.For_i hardware
    loops, so the NEFF stays ~1.5K instructions regardless of reps.
  - W-gen matmuls run as float32r (full fp32 data, 4x PE rate).
  - PSUM evacuation of the per-edge weight tensor runs on ACT (Prelu,
    bf16 out); the per-edge contraction is a broadcast-multiply +
    segmented reduce on DVE (2 ops per tile).
"""
import contextlib
import sys

sys.path.insert(0, "/opt/trn_rl_repo")

import numpy as np

import concourse.bass as bass
import concourse.tile as tile
from concourse import bacc, mybir
from concourse.bass import ds
from concourse.bass_utils import run_bass_kernel_spmd
from concourse.masks import make_identity

F32 = mybir.dt.float32
F32R = mybir.dt.float32r
BF16 = mybir.dt.bfloat16
I32 = mybir.dt.int32
AF = mybir.ActivationFunctionType
OP = mybir.AluOpType

N_CORES = 8
N = 8000
E = 32000
G = 32
NP = 8192            # padded nodes (64 * 128)
NT = NP // 128       # 64 node tiles
K = 5                # edge tiles per node tile
ET = NT * K          # 320 edge tiles
EP = ET * 128        # 40960 edge slots
JUNK = NP - 1        # junk node for pad-edge sources

LAYERS = [(4, 8), (8, 64), (64, 128)]


def build_graph(reps: int = 1, debug: bool = False):
    nc = bacc.Bacc("TRN2", target_bir_lowering=False, debug=False,
                   num_devices=N_CORES)

    def din(name, shape, dt=F32):
        return nc.dram_tensor(name, shape, dt, kind="ExternalInput").ap()

    v = {}
    # per-node-tile bf16 stream: [xsrc(20) | dstloc(5) | onehot(32)] = 57 cols
    SW = 57
    v["strm"] = din("strm", [128, NT * SW], BF16)
    v["ea"] = din("ea", [4, EP])                 # edge attrs^T + ones row
    v["srcidx"] = din("srcidx", [128, ET], I32)  # src ids, tile t in col t
    v["xaT"] = din("xaT", [5, NP])               # x^T + ones row
    v["w1a"] = [din(f"w1a{l}", [4, 64]) for l in range(3)]
    v["w2a"] = [din(f"w2a{l}", [65, LAYERS[l][0] * LAYERS[l][1]], F32R)
                for l in range(3)]
    v["r_a"] = [din("r1a", [5, 8]), din("r2a", [9, 64]), din("r3a", [65, 128])]
    v["fc1"] = din("fc1", [128, 128])
    v["fc1b"] = din("fc1b", [128, 1])
    v["fc2"] = din("fc2", [128, 64])
    v["fc2b"] = din("fc2b", [64, 1])
    v["fc3"] = din("fc3", [64, 1])
    v["fc3b"] = din("fc3b", [G, 1])

    v["out"] = nc.dram_tensor("out", [G, 1], F32, kind="ExternalOutput").ap()
    dbg = {}
    if debug:
        dbg["h1"] = nc.dram_tensor("h1_o", [NP, 8], F32, kind="ExternalOutput").ap()
        dbg["h2"] = nc.dram_tensor("h2_o", [NP, 64], F32, kind="ExternalOutput").ap()
        dbg["pool"] = nc.dram_tensor("pool_o", [128, G], F32, kind="ExternalOutput").ap()

    # internal DRAM
    v["h_dram"] = [None,
                   nc.dram_tensor("h1d", [NP, 8], BF16).ap(),
                   nc.dram_tensor("h2d", [NP, 64], BF16).ap()]
    v["hpT_dram"] = [None,
                     nc.dram_tensor("h1pTd", [9, NP], F32).ap(),
                     nc.dram_tensor("h2pTd", [65, NP], F32).ap()]

    with tile.TileContext(nc) as tc:
        _build_body(nc, tc, reps, v, dbg)
    nc.compile()
    return nc


def _build_body(nc, tc, reps, v, dbg):
    ctx = contextlib.ExitStack()
    with ctx:
        persist = ctx.enter_context(tc.tile_pool(name="persist", bufs=1))
        psum_w = ctx.enter_context(tc.tile_pool(name="psum_w", bufs=3, space="PSUM"))
        psum_a = ctx.enter_context(tc.tile_pool(name="psum_a", bufs=1, space="PSUM"))
        psum_m = ctx.enter_context(tc.tile_pool(name="psum_m", bufs=1, space="PSUM"))

        # ---- persistent SBUF ----
        ident = persist.tile([128, 128], F32, name="ident")
        make_identity(nc, ident[:])
        w1a_sb, w2a_sb, ra_sb = [], [], []
        for l in range(3):
            t1 = persist.tile([4, 64], F32, name=f"w1a_sb{l}")
            nc.sync.dma_start(t1[:], v["w1a"][l][:])
            w1a_sb.append(t1)
            c = LAYERS[l][0] * LAYERS[l][1]
            t2 = persist.tile([65, c], F32R, name=f"w2a_sb{l}")
            nc.sync.dma_start(t2[:], v["w2a"][l][:])
            w2a_sb.append(t2)
            kd = LAYERS[l][0] + 1
            t3 = persist.tile([kd, LAYERS[l][1]], F32, name=f"ra_sb{l}")
            nc.sync.dma_start(t3[:], v["r_a"][l][:])
            ra_sb.append(t3)
        fc1_sb = persist.tile([128, 128], F32, name="fc1_sb")
        nc.sync.dma_start(fc1_sb[:], v["fc1"][:])
        fc1b_sb = persist.tile([128, 1], F32, name="fc1b_sb")
        nc.sync.dma_start(fc1b_sb[:], v["fc1b"][:])
        fc2_sb = persist.tile([128, 64], F32, name="fc2_sb")
        nc.sync.dma_start(fc2_sb[:], v["fc2"][:])
        fc2b_sb = persist.tile([64, 1], F32, name="fc2b_sb")
        nc.sync.dma_start(fc2b_sb[:], v["fc2b"][:])
        fc3_sb = persist.tile([64, 1], F32, name="fc3_sb")
        nc.sync.dma_start(fc3_sb[:], v["fc3"][:])
        fc3b_sb = persist.tile([G, 1], F32, name="fc3b_sb")
        nc.sync.dma_start(fc3b_sb[:], v["fc3b"][:])

        # working buffers, two sets (loop body handles 2 node tiles)
        ea_sb = [persist.tile([4, 640], F32, name=f"ea_sb{b}") for b in range(2)]
        efT = [persist.tile([65, 640], F32, name=f"efT{b}") for b in range(2)]
        nc.vector.memset(efT[0][64:65, :], 1.0)
        nc.vector.memset(efT[1][64:65, :], 1.0)
        efT_r = [persist.tile([65, 640], F32R, name=f"efT_r{b}") for b in range(2)]
        strm_sb = [persist.tile([128, 57], BF16, name=f"strm_sb{b}") for b in range(2)]
        pmat_sb = [persist.tile([128, 640], BF16, name=f"pmat_sb{b}") for b in range(2)]
        iota_i = persist.tile([128, 128], I32, name="iota_i")
        nc.gpsimd.iota(iota_i[:], pattern=[[1, 128]], channel_multiplier=0)
        iota_b = persist.tile([128, 128], BF16, name="iota_b")
        nc.vector.tensor_copy(iota_b[:], iota_i[:])
        sidx_sb = [persist.tile([128, K], I32, name=f"sidx_sb{b}") for b in range(2)]
        dst_f = [persist.tile([128, K], F32, name=f"dst_f{b}") for b in range(2)]
        hpT_sb = [persist.tile([65, 128], F32, name=f"hpT_sb{b}") for b in range(2)]
        h_gath = [persist.tile([128, K * 64], BF16, name=f"h_gath{b}") for b in range(2)]
        w_sb4 = [[persist.tile([128, 8192], BF16, name=f"w_sb{b}_{t}")
                  for t in range(2)] for b in range(2)]
        wh2 = [persist.tile([128, 8192], BF16, name=f"wh{b}")
               for b in range(2)]
        acc5 = [[persist.tile([128, 128], BF16, name=f"acc{b}_{t}")
                 for t in range(K)] for b in range(2)]
        hj = [persist.tile([128, 128], F32, name=f"hj{b}") for b in range(2)]
        hjb = [persist.tile([128, 128], BF16, name=f"hjb{b}") for b in range(2)]
        hT_sb = [persist.tile([65, 128], F32, name=f"hT_sb{b}") for b in range(2)]
        pool_sb = persist.tile([128, G], F32, name="pool_sb")
        mlp_t1 = persist.tile([128, G], F32, name="mlp_t1")
        mlp_t2 = persist.tile([64, G], F32, name="mlp_t2")
        mlp_ot = persist.tile([G, 1], F32, name="mlp_ot")

        h_dram, hpT_dram = v["h_dram"], v["hpT_dram"]

        def edge_tile(l, b, tt):
            """W-gen + chunk-fused MAC for one edge tile (o-major W)."""
            cin, cout = LAYERS[l]
            ncols = cin * cout
            w_sb = w_sb4[b][tt % 2]
            wh = wh2[b]
            acc = acc5[b][tt]
            ef_sl = efT_r[b][:, 128 * tt:128 * (tt + 1)]
            if l == 0:
                hsl = strm_sb[b][:, 4 * tt:4 * tt + cin]
            else:
                hsl = h_gath[b][:, 64 * tt:64 * tt + cin]
            # W-gen in 1024-col PSUM groups; the MAC consumes every 2048-col
            # region as it lands (mult + segmented reduce over its o-range).
            for gb in range(0, ncols, 1024):
                gcols = min(1024, ncols - gb)
                ps = psum_w.tile([128, 1024], F32, name="wps", tag="wps")
                for sb in range(0, gcols, 512):
                    scols = min(512, gcols - sb)
                    nc.tensor.matmul(ps[:, sb:sb + scols],
                                     lhsT=ef_sl,
                                     rhs=w2a_sb[l][:, gb + sb:gb + sb + scols],
                                     start=True, stop=True)
                nc.scalar.activation(w_sb[:, gb:gb + gcols], ps[:, :gcols],
                                     AF.Prelu, alpha=0.1)
                me = gb + gcols
                if me == ncols or me % 2048 == 0:
                    mb = (me - 1) // 2048 * 2048
                    o0, o1 = mb // cin, me // cin
                    mc = me - mb
                    w3 = w_sb[:, mb:me].rearrange("p (o i) -> p o i", i=cin)
                    h3 = hsl.rearrange("p (o i) -> p o i", o=1) \
                            .broadcast_to([128, o1 - o0, cin])
                    nc.vector.tensor_tensor(
                        wh[:, mb:me].rearrange("p (o i) -> p o i", i=cin),
                        w3, h3, op=OP.mult)
                    with nc.allow_low_precision("single rounding of fp32 sum"):
                        nc.vector.tensor_reduce(
                            acc[:, o0:o1],
                            wh[:, mb:me].rearrange("p (o i) -> p o i", i=cin),
                            axis=mybir.AxisListType.X, op=OP.add)

        def sub_loads(l, b, j):
            """All DMAs for sub-tile j (issued on Pool queue)."""
            kd = LAYERS[l][0] + 1
            cin = LAYERS[l][0]
            nc.sync.dma_start(strm_sb[b][:], v["strm"][:, ds(57 * j, 57)])
            nc.sync.dma_start(ea_sb[b][:], v["ea"][:, ds(640 * j, 640)])
            if l == 0:
                nc.sync.dma_start(hpT_sb[b][:kd, :],
                                    v["xaT"][:, ds(128 * j, 128)])
            else:
                nc.sync.dma_start(sidx_sb[b][:],
                                    v["srcidx"][:, ds(K * j, K)])
                nc.sync.dma_start(hpT_sb[b][:kd, :],
                                    hpT_dram[l][:, ds(128 * j, 128)])
                for tt in range(K):
                    nc.gpsimd.indirect_dma_start(
                        out=h_gath[b][:, 64 * tt:64 * tt + cin],
                        out_offset=None,
                        in_=h_dram[l][:],
                        in_offset=bass.IndirectOffsetOnAxis(
                            ap=sidx_sb[b][:, tt:tt + 1], axis=0),
                    )

        def sub_compute(l, b, j):
            cin, cout = LAYERS[l]
            kd = cin + 1
            # build the 5 aggregation one-hot matrices from dstloc
            nc.vector.tensor_copy(dst_f[b][:], strm_sb[b][:, 20:25])
            for tt in range(K):
                nc.vector.tensor_scalar(
                    pmat_sb[b][:, 128 * tt:128 * (tt + 1)], iota_b[:],
                    dst_f[b][:, tt:tt + 1], None, op0=OP.is_equal)

            # edge-net hidden for the 5 tiles: efT = Prelu(w1a^T @ ea)
            for cs in range(0, 640, 512):
                cw = min(512, 640 - cs)
                psf = psum_m.tile([64, 512], F32, name="efps", tag="psm")
                nc.tensor.matmul(psf[:, :cw], lhsT=w1a_sb[l][:],
                                 rhs=ea_sb[b][:, cs:cs + cw],
                                 start=True, stop=True)
                nc.scalar.activation(efT[b][0:64, cs:cs + cw], psf[:, :cw],
                                     AF.Prelu, alpha=0.1)
            nc.vector.tensor_copy(efT_r[b][:], efT[b][:])

            # edge tiles: W-gen + MAC (PE runs all W-gens first)
            for tt in range(K):
                edge_tile(l, b, tt)
            # aggregation + root into one PSUM group
            agg_ps = psum_a.tile([128, 512], F32, name="agg_ps", tag="agg")
            for tt in range(K):
                nc.tensor.matmul(agg_ps[:, :cout],
                                 lhsT=pmat_sb[b][:, 128 * tt:128 * (tt + 1)],
                                 rhs=acc5[b][tt][:, :cout],
                                 start=(tt == 0), stop=False)
            # root term accumulated into the same PSUM group
            nc.tensor.matmul(agg_ps[:, :cout], lhsT=hpT_sb[b][:kd, :],
                             rhs=ra_sb[l][:], start=False, stop=True)

            # node update
            nc.scalar.activation(hj[b][:, :cout], agg_ps[:, :cout],
                                 AF.Prelu, alpha=0.1)
            if l < 2:
                nc.vector.tensor_copy(hjb[b][:, :cout], hj[b][:, :cout])
                nc.sync.dma_start(h_dram[l + 1][ds(128 * j, 128), :],
                                    hjb[b][:, :cout])
                nc.vector.memset(hj[b][:, cout:cout + 1], 1.0)
                tps = psum_m.tile([128, 512], F32, name="tps", tag="psm")
                nc.tensor.transpose(tps[:cout + 1, :128],
                                    hj[b][:, :cout + 1], ident[:])
                nc.vector.tensor_copy(hT_sb[b][:cout + 1, :],
                                      tps[:cout + 1, :128])
                nc.sync.dma_start(hpT_dram[l + 1][:, ds(128 * j, 128)],
                                    hT_sb[b][:cout + 1, :])
            else:
                nc.vector.tensor_copy(hjb[b][:, :cout], hj[b][:, :cout])
                pps = psum_m.tile([128, 512], F32, name="pps", tag="psm")
                nc.tensor.matmul(pps[:, :G], lhsT=hjb[b][:, :cout],
                                 rhs=strm_sb[b][:, 25:57], start=True,
                                 stop=True)
                nc.vector.tensor_tensor(pool_sb[:], pool_sb[:],
                                        pps[:, :G], op=OP.add)

        def layer_loop(l):
            with tc.For_i(0, NT // 2) as jh:
                for b in range(2):
                    sub_loads(l, b, 2 * jh + b)
                for b in range(2):
                    sub_compute(l, b, 2 * jh + b)

        rep_ctx = tc.For_i(0, reps)
        with rep_ctx:
            nc.vector.memset(pool_sb[:], 0.0)
            for l in range(3):
                layer_loop(l)

            if "pool" in dbg:
                nc.sync.dma_start(dbg["pool"][:], pool_sb[:])
            # ---- MLP (transposed) ----
            m1 = psum_m.tile([128, 512], F32, name="m1", tag="psm")
            nc.tensor.matmul(m1[:, :G], lhsT=fc1_sb[:], rhs=pool_sb[:],
                             start=True, stop=True)
            nc.scalar.activation(mlp_t1[:], m1[:, :G], AF.Prelu,
                                 bias=fc1b_sb[:, :1], alpha=0.1)
            m2 = psum_m.tile([128, 512], F32, name="m2", tag="psm")
            nc.tensor.matmul(m2[:64, :G], lhsT=fc2_sb[:], rhs=mlp_t1[:],
                             start=True, stop=True)
            nc.scalar.activation(mlp_t2[:], m2[:64, :G], AF.Prelu,
                                 bias=fc2b_sb[:, :1], alpha=0.1)
            m3 = psum_m.tile([128, 512], F32, name="m3", tag="psm")
            nc.tensor.matmul(m3[:G, :1], lhsT=mlp_t2[:], rhs=fc3_sb[:],
                             start=True, stop=True)
            nc.vector.scalar_tensor_tensor(out=mlp_ot[:], in0=m3[:G, :1],
                                           scalar=1.0, in1=fc3b_sb[:],
                                           op0=OP.mult, op1=OP.add)
            nc.sync.dma_start(v["out"][:], mlp_ot[:])

        if "h1" in dbg:
            snb = persist.tile([128, 64], BF16, name="dbg_snb")
            sn = persist.tile([128, 64], F32, name="dbg_sn")
            for jj in range(NT):
                nc.sync.dma_start(snb[:, :8], h_dram[1][128 * jj:128 * (jj + 1), :])
                nc.vector.tensor_copy(sn[:, :8], snb[:, :8])
                nc.sync.dma_start(dbg["h1"][128 * jj:128 * (jj + 1), :], sn[:, :8])
                nc.sync.dma_start(snb[:, :64], h_dram[2][128 * jj:128 * (jj + 1), :])
                nc.vector.tensor_copy(sn[:, :64], snb[:, :64])
                nc.sync.dma_start(dbg["h2"][128 * jj:128 * (jj + 1), :], sn[:, :64])


# ---------------------------------------------------------------------------
# host side
# ---------------------------------------------------------------------------

_CACHE = {}


def _prep_inputs(x, edge_index, batch_index, p):
    """Build the replicated input map (same for every core)."""
    xs = x.astype(np.float32)
    src_all = edge_index[0].astype(np.int64)
    dst_all = edge_index[1].astype(np.int64)

    # pack edges: node tile j gets slots [640j, 640j+cnt)
    ea = np.zeros((4, EP), np.float32)
    ea[3, :] = 1.0
    srcidx = np.full((EP,), JUNK, np.int64)
    dstloc = np.full((EP,), -1, np.int64)                   # local dst or -1
    eattr_all = (xs[dst_all] - xs[src_all])[:, 1:]          # [E, 3]

    tile_of_dst = dst_all // 128
    order = np.argsort(tile_of_dst, kind="stable")
    counts = np.bincount(tile_of_dst, minlength=NT)
    assert counts.max() <= K * 128, f"node tile overflow: {counts.max()}"
    starts = np.zeros(NT + 1, np.int64)
    starts[1:] = np.cumsum(counts)
    for j in range(NT):
        idx = order[starts[j]:starts[j + 1]]                 # edges of tile j
        n = len(idx)
        slots = 640 * j + np.arange(n)
        srcidx[slots] = src_all[idx]
        dstloc[slots] = dst_all[idx] - 128 * j
        ea[:3, slots] = eattr_all[idx].T

    srcidx_t = srcidx.reshape(ET, 128).T.astype(np.int32)    # [128, ET]
    xsrc = np.zeros((128, ET * 4), np.float32)
    xs_pad = np.zeros((NP, 4), np.float32)
    xs_pad[:N] = xs
    for t in range(ET):
        xsrc[:, 4 * t:4 * t + 4] = xs_pad[srcidx.reshape(ET, 128)[t]]

    import ml_dtypes
    bf16 = ml_dtypes.bfloat16

    onehot = np.zeros((128, NT * G), np.float32)
    bi = batch_index.astype(np.int64)
    for jj in range(NT):
        nodes = np.arange(128 * jj, 128 * jj + 128)
        real = nodes < N
        onehot[real, G * jj + bi[nodes[real]]] = 1.0

    # bf16 per-node-tile stream: [xsrc(20) | dstloc(5) | onehot(32)]
    strm = np.zeros((128, NT * 57), np.float32)
    dst_rt = dstloc.reshape(ET, 128)
    for j in range(NT):
        strm[:, 57 * j:57 * j + 20] = xsrc[:, 20 * j:20 * (j + 1)]
        for tt in range(K):
            strm[:, 57 * j + 20 + tt] = dst_rt[K * j + tt]
        strm[:, 57 * j + 25:57 * (j + 1)] = onehot[:, G * j:G * (j + 1)]

    xaT = np.zeros((5, NP), np.float32)
    xaT[:4, :N] = xs.T
    xaT[4, :] = 1.0

    def aug_w(w, b):
        return np.vstack([w, b[None, :]]).astype(np.float32)

    def om(w2, l):
        """Reorder edge-net output cols from i-major to o-major."""
        cin, cout = LAYERS[l]
        return np.ascontiguousarray(
            w2.reshape(-1, cin, cout).transpose(0, 2, 1).reshape(-1, cin * cout))

    return dict(
        ea=ea, strm=strm.astype(bf16), srcidx=srcidx_t, xaT=xaT,
        w1a0=aug_w(p["en1_w1"], p["en1_b1"]),
        w2a0=om(aug_w(p["en1_w2"], p["en1_b2"]), 0),
        w1a1=aug_w(p["en2_w1"], p["en2_b1"]),
        w2a1=om(aug_w(p["en2_w2"], p["en2_b2"]), 1),
        w1a2=aug_w(p["en3_w1"], p["en3_b1"]),
        w2a2=om(aug_w(p["en3_w2"], p["en3_b2"]), 2),
        r1a=aug_w(p["root1"], p["cb1"]), r2a=aug_w(p["root2"], p["cb2"]),
        r3a=aug_w(p["root3"], p["cb3"]),
        fc1=p["fc1_w"].astype(np.float32),
        fc1b=p["fc1_b"].reshape(128, 1).astype(np.float32),
        fc2=p["fc2_w"].astype(np.float32),
        fc2b=p["fc2_b"].reshape(64, 1).astype(np.float32),
        fc3=p["fc3_w"].astype(np.float32),
        fc3b=np.repeat(p["fc3_b"].reshape(1, 1), G, 0).astype(np.float32),
    )


def kernel(x, edge_index, batch_index, **p):
    if "nc" not in _CACHE:
        _CACHE["nc"] = build_graph(reps=1)
    nc = _CACHE["nc"]
    in_map = _prep_inputs(np.asarray(x), np.asarray(edge_index),
                          np.asarray(batch_index),
                          {k: np.asarray(v) for k, v in p.items()})
    in_maps = [in_map for _ in range(N_CORES)]
    res = run_bass_kernel_spmd(nc, in_maps, list(range(N_CORES)))
    return res.results[0]["out"].astype(np.float32)
